# revision 12
# baseline (speedup 1.0000x reference)
"""LinearAttention (relu feature map) + residual + LayerNorm on 8 TRN2 cores.

Reference (per batch b):
  q = relu(x @ Wq.T + bq); k = relu(x @ Wk.T + bk); v = x @ Wv.T + bv
  kv[h] = sum_n k[n,h,:] outer v[n,h,:];  k_sum[h] = sum_n k[n,h,:]
  denom = max(q . k_sum, 1e-6); ctx = q @ kv
  y = ctx/denom + x; out = LayerNorm(y) * gamma + beta

Sharding: core c handles (b = c//2, token half = c%2) -> T=2048 tokens.
kv/k_sum are partial sums over the core's tokens; a pairwise AllReduce
([0,1],[2,3],...) merges them. Everything else is core-local.

Dtypes: the three projections run as fp8e4 DoubleRow matmuls (256-deep
contraction per pass, 2x PE rate). Host pre-scales x by 16 and W.T by
256 so fp8 operands sit in the normal range; the 1/4096 unscale is
fused into the psum eviction. k/v/q are stored bf16; kv and ctx run as
bf16 matmuls (full rate at any free dim, unlike fp32r's 4x penalty
below free=256). The attention path feeds y = attn + x where attn is
~50x smaller than x, so fp8 noise is diluted far below the 2e-2 gate.

Schedule: k/v proj (PE-bound, ~60us) -> kv in PSUM chains -> AllReduce
emitted immediately, with the whole q projection (LDW-amortized: one
stationary load per 8 matmuls) running under it -> ctx blocks. The
denominator is folded into the ctx matmul: kvbd carries two extra
columns with k_sum per head, so each pair emits [ctx | den] in one
pass. Several chains share a PSUM bank (start=True zeroes the whole
2KB bank; later chains in the bank use start=False and ride on that
zeroing - safe because the PE runs its queue in program order).
"""
import numpy as np
import ml_dtypes

import concourse.bass as bass
import concourse.tile as tile
from concourse import bacc, mybir
from concourse.bass_utils import run_bass_kernel_spmd
from concourse.bass import ts

B, NTOK, DIM, H, HD = 4, 4096, 1024, 16, 64
T = 2048          # tokens per core
P = 128           # partitions
C2 = 4            # fp8 DoubleRow contraction double-chunks (256 ch each)
TT = 16           # 128-token tiles per core
NPAIR = 8         # head pairs (one per 128-channel chunk)
PW = 130          # pair width: 128 ctx cols + 2 den/ksum cols
F2 = 512          # phase-2 token tile
SX = 16.0         # host scale on x for fp8
SW = 256.0        # host scale on W.T for fp8
UNSCALE = 1.0 / (SX * SW)
EPS_DENOM = 1e-6
EPS_LN = 1e-5
N_CORES = 8

F32 = mybir.dt.float32
BF16 = mybir.dt.bfloat16
F8 = mybir.dt.float8e4
AF = mybir.ActivationFunctionType
ALU = mybir.AluOpType
DR = mybir.MatmulPerfMode.DoubleRow


def build(with_kv_bias: bool = False, with_affine: bool = False) -> "bacc.Bacc":
    nc = bacc.Bacc("TRN2", target_bir_lowering=False, debug=False,
                   num_devices=N_CORES)

    xt8_in = nc.dram_tensor("xt8", [DIM, T], F8, kind="ExternalInput").ap()
    xn_in = nc.dram_tensor("xn", [T, DIM], BF16, kind="ExternalInput").ap()
    wq8_in = nc.dram_tensor("wq8", [DIM, DIM], F8, kind="ExternalInput").ap()
    wk8_in = nc.dram_tensor("wk8", [DIM, DIM], F8, kind="ExternalInput").ap()
    wv8_in = nc.dram_tensor("wv8", [DIM, DIM], F8, kind="ExternalInput").ap()
    bq_in = nc.dram_tensor("bq", [DIM], F32, kind="ExternalInput").ap()
    bk_in = nc.dram_tensor("bk", [DIM], BF16, kind="ExternalInput").ap()
    bv_in = nc.dram_tensor("bv", [DIM], BF16, kind="ExternalInput").ap()
    gamma_in = nc.dram_tensor("gamma", [DIM], BF16, kind="ExternalInput").ap()
    beta_in = nc.dram_tensor("beta", [DIM], BF16, kind="ExternalInput").ap()
    yn_out = nc.dram_tensor("yn", [T, DIM], BF16, kind="ExternalOutput").ap()

    def bcast_dram_row(ap, n):
        # DRAM [D] -> [[0,n],[1,D]] so DMA replicates the row to n partitions
        return bass.AP(tensor=ap.tensor, offset=ap.offset,
                       ap=[[0, n]] + list(ap.ap))

    with tile.TileContext(nc) as tc:
        with (
            tc.tile_pool(name="persist", bufs=1) as persist,
            tc.tile_pool(name="dram", bufs=2, space="DRAM") as dram,
        ):
            # fp8 operands: [part=inner128, c2, two, free] so DoubleRow can
            # slice [128, 2, free] with dim1 = the 2 contraction chunks.
            # DMA order = need order: per-c2 w chunks, then x token groups.
            wk8_sb = persist.tile([P, C2, 2, DIM], F8)
            wv8_sb = persist.tile([P, C2, 2, DIM], F8)
            wk8_r = wk8_in.rearrange("(c2 two p) o -> p c2 two o", p=P, two=2)
            wv8_r = wv8_in.rearrange("(c2 two p) o -> p c2 two o", p=P, two=2)
            xt8_sb = persist.tile([P, C2, 2, T], F8)
            xt8_r = xt8_in.rearrange("(c2 two p) t -> p c2 two t", p=P, two=2)
            # interleave so the first tile's operands land soonest
            for c2 in range(C2):
                nc.sync.dma_start(wk8_sb[:, c2], wk8_r[:, c2])
                nc.sync.dma_start(wv8_sb[:, c2], wv8_r[:, c2])
                nc.sync.dma_start(xt8_sb[:, :, :, ts(c2, T // 4)],
                                  xt8_r[:, :, :, ts(c2, T // 4)])
            wq8_sb = persist.tile([P, C2, 2, DIM], F8)
            nc.sync.dma_start(
                wq8_sb[:], wq8_in.rearrange("(c2 two p) o -> p c2 two o",
                                            p=P, two=2))

            bq_sb = persist.tile([P, NPAIR], F32)
            nc.sync.dma_start(bq_sb[:], bq_in.rearrange("(co p) -> p co", p=P))
            if with_affine:
                gamma_bc = persist.tile([P, DIM], BF16)
                nc.sync.dma_start(gamma_bc[:], bcast_dram_row(gamma_in, P))
                beta_bc = persist.tile([P, DIM], BF16)
                nc.sync.dma_start(beta_bc[:], bcast_dram_row(beta_in, P))
            if with_kv_bias:
                bk_bc = persist.tile([P, DIM], BF16)
                nc.sync.dma_start(bk_bc[:], bcast_dram_row(bk_in, P))
                bv_bc = persist.tile([P, DIM], BF16)
                nc.sync.dma_start(bv_bc[:], bcast_dram_row(bv_in, P))
            eps_sb = persist.tile([P, 1], F32)
            nc.vector.memset(eps_sb[:], EPS_LN)

            # k/v stored fp8 at 16x true scale so kv runs as DoubleRow too;
            # the 256x on kv/ksum psum is unscaled during compaction
            k_sb = persist.tile([P, TT, DIM], F8)
            v_sb = persist.tile([P, TT, NPAIR * PW], F8)
            # ones columns (=16 to match the k/v scale) ride along in v so
            # k_sum falls out of the kv matmul
            nc.vector.memset(
                v_sb[:].rearrange("p t (pr c) -> p t pr c", c=PW)[:, :, :, P:],
                SX)

            qt_sb = persist.tile([P, NPAIR, T], BF16)
            kvbd = persist.tile([P, NPAIR, PW], BF16)
            kv_send = persist.tile([P, NPAIR * HD + NPAIR], F32)  # [128, 520]
            kv_red = persist.tile([P, NPAIR * HD + NPAIR], F32)

            # Warm up the CC collective library under phase 1a so the real
            # AllReduce doesn't pay the ~11us LIBRARY_RELOAD on its path.
            warm_in = dram.tile([1, 2], F32)
            warm_out = dram.tile([1, 2], F32)
            warm_sb = persist.tile([1, 2], F32)
            nc.vector.memset(warm_sb[:], 0.0)
            nc.sync.dma_start(warm_in[:], warm_sb[:])
            nc.gpsimd.collective_compute(
                "AllReduce", ALU.add,
                replica_groups=[[0, 1], [2, 3], [4, 5], [6, 7]],
                ins=[warm_in.opt()], outs=[warm_out.opt()])

            # ---------------- Phase 1a: k, v projections (fp8 DoubleRow) ----
            # Per tile: stationary x chunk is reused across 8 moving w blocks
            # (4 oc each for k and v). psum [128,2,512] = 2 banks; the two
            # 256-wide chains per bank share its start=True zeroing.
            with tc.tile_pool(name="p1psum", bufs=2, space="PSUM") as p1psum:
                for i in range(TT):
                    kps = p1psum.tile([P, 2, F2], F32, tag="kps")
                    vps = p1psum.tile([P, 2, F2], F32, tag="vps")
                    for c2 in range(C2):
                        xs = xt8_sb[:, c2, :, ts(i, P)]
                        for ps, w8 in ((kps, wk8_sb), (vps, wv8_sb)):
                            for oc in range(4):
                                bk_, h = divmod(oc, 2)
                                nc.tensor.matmul(
                                    ps[:, bk_, ts(h, 256)], xs,
                                    w8[:, c2, :, ts(oc, 256)],
                                    start=(c2 == 0 and h == 0),
                                    stop=(c2 == C2 - 1),
                                    perf_mode=DR)
                    for bk_ in range(2):
                        # k: relu + rescale to 16x fp8 fused into the
                        # eviction (scalar); bias fallback needs pre-relu
                        nc.scalar.activation(k_sb[:, i, ts(bk_, F2)],
                                             kps[:, bk_, :],
                                             AF.Copy if with_kv_bias
                                             else AF.Relu,
                                             scale=SX * UNSCALE)
                        # v: rescale on vector, strided into 130-wide pairs
                        dst = v_sb[:, i, bk_ * 4 * PW:(bk_ + 1) * 4 * PW] \
                            .rearrange("p (pr c) -> p pr c", c=PW)[:, :, 0:P]
                        src = vps[:, bk_, :].rearrange("p (pr c) -> p pr c",
                                                       c=P)
                        nc.vector.tensor_scalar(dst, src, SX * UNSCALE, None,
                                                op0=ALU.mult)
                    if with_kv_bias:
                        # general fallback (unused for this problem's zero
                        # biases): add bias then relu k in place
                        nc.vector.tensor_add(k_sb[:, i, :], k_sb[:, i, :],
                                             bk_bc[:])
                        nc.vector.tensor_scalar_max(k_sb[:, i, :],
                                                    k_sb[:, i, :], 0.0)
                        dstv = v_sb[:, i, :].rearrange(
                            "p (pr c) -> p pr c", c=PW)[:, :, 0:P]
                        nc.vector.tensor_add(
                            dstv, dstv,
                            bv_bc[:].rearrange("p (pr c) -> p pr c", c=P))

            # ---------------- Phase 1b: kv + k_sum in PSUM chains -----------
            # fp8 DoubleRow over pairs of token tiles (256-token contraction)
            with tc.tile_pool(name="kvpsum", bufs=1, space="PSUM") as kvpool:
                kvps = kvpool.tile([P, NPAIR, F2], F32)  # 8 banks
                for p in range(NPAIR):
                    for i2 in range(TT // 2):
                        nc.tensor.matmul(
                            kvps[:, p, 0:PW],
                            k_sb[:, 2 * i2:2 * i2 + 2, ts(p, P)],
                            v_sb[:, 2 * i2:2 * i2 + 2, p * PW:(p + 1) * PW],
                            start=(i2 == 0), stop=(i2 == TT // 2 - 1),
                            perf_mode=DR)
                # compact evict (with the 1/256 unscale): diagonal 64x64
                # blocks + ksum col -> kv_send
                KVUS = 1.0 / (SX * SX)  # k,v carry 16x each
                nc.vector.tensor_scalar(
                    kv_send[0:HD, 0:NPAIR * HD].rearrange(
                        "p (n d) -> p n d", d=HD),
                    kvps[0:HD, :, 0:HD], KVUS, None, op0=ALU.mult)
                nc.vector.tensor_scalar(
                    kv_send[HD:P, 0:NPAIR * HD].rearrange(
                        "p (n d) -> p n d", d=HD),
                    kvps[HD:P, :, HD:P], KVUS, None, op0=ALU.mult)
                nc.vector.tensor_scalar(
                    kv_send[:, NPAIR * HD:].rearrange(
                        "p (n o) -> p n o", o=1),
                    kvps[:, :, P:P + 1], KVUS, None, op0=ALU.mult)

            # ---- AllReduce kv/k_sum across token-half pairs (emitted
            # outside any psum scope so pool turnover can't delay it) ----
            cc_in = dram.tile([P, NPAIR * HD + NPAIR], F32)
            cc_out = dram.tile([P, NPAIR * HD + NPAIR], F32)
            nc.sync.dma_start(cc_in[:], kv_send[:])
            nc.gpsimd.collective_compute(
                "AllReduce", ALU.add,
                replica_groups=[[0, 1], [2, 3], [4, 5], [6, 7]],
                ins=[cc_in.opt()], outs=[cc_out.opt()])
            nc.sync.dma_start(kv_red[:], cc_out[:])

            # ---------------- Phase 2: q proj, ctx|den, residual, LN --------
            # q runs W-stationary in two 1024-token groups (one stationary
            # load per 4 matmuls) with ctx blocks interleaved between them,
            # so the vector-bound postprocess starts as soon as the AllReduce
            # lands while the PE finishes the second q group.
            with (
                tc.tile_pool(name="qpsum", bufs=2, space="PSUM") as qpsum,
                tc.tile_pool(name="ctxpsum", bufs=1, space="PSUM") as ctxpsum,
                tc.tile_pool(name="work", bufs=4) as work,
                tc.tile_pool(name="small", bufs=6) as small,
            ):
                def qproj_group(g):
                    # tokens [g*1024, (g+1)*1024) = tb blocks 4g..4g+3
                    for co in range(NPAIR):
                        qp = qpsum.tile([P, 4, 256], F32, tag="qps")
                        for c2 in range(C2):
                            for tb in range(4):
                                nc.tensor.matmul(
                                    qp[:, tb, :],
                                    wq8_sb[:, c2, :, ts(co, P)],
                                    xt8_sb[:, c2, :, ts(4 * g + tb, 256)],
                                    start=(c2 == 0 and tb % 2 == 0),
                                    stop=(c2 == C2 - 1),
                                    perf_mode=DR)
                        # relu(q/4096 + bq) fused into the psum eviction
                        nc.scalar.activation(
                            qt_sb[:, co, ts(g, T // 2)],
                            qp[:].rearrange("p a b -> p (a b)"),
                            AF.Relu, scale=UNSCALE, bias=bq_sb[:, co:co + 1])

                qproj_group(0)

                # rebuild block-diag kv/ksum operand (bf16) from the reduce
                nc.vector.memset(kvbd[:], 0.0)
                nc.vector.tensor_copy(
                    kvbd[0:HD, :, 0:HD],
                    kv_red[0:HD, 0:NPAIR * HD].rearrange("p (n d) -> p n d",
                                                         d=HD))
                nc.vector.tensor_copy(
                    kvbd[HD:P, :, HD:P],
                    kv_red[HD:P, 0:NPAIR * HD].rearrange("p (n d) -> p n d",
                                                         d=HD))
                nc.vector.tensor_copy(
                    kvbd[0:HD, :, P:P + 1],
                    kv_red[0:HD, NPAIR * HD:].rearrange("p (n o) -> p n o",
                                                        o=1))
                nc.vector.tensor_copy(
                    kvbd[HD:P, :, P + 1:P + 2],
                    kv_red[HD:P, NPAIR * HD:].rearrange("p (n o) -> p n o",
                                                        o=1))

                for blk in range(T // P):
                    if blk == 6:
                        qproj_group(1)
                    t0 = blk * P
                    # ctx|den: pairs packed 3/bank (520B each) so the
                    # eviction reads regular strides; pairs after the first
                    # in a bank ride its start=True zeroing
                    cps = ctxpsum.tile([P, 3, F2], F32, tag="ctx")
                    for p in range(NPAIR):
                        bk_, pr = divmod(p, 3)
                        nc.tensor.matmul(
                            cps[:, bk_, pr * PW:(pr + 1) * PW],
                            qt_sb[:, p, t0:t0 + P], kvbd[:, p, :],
                            start=(pr == 0),
                            stop=(pr == 2 or p == NPAIR - 1))

                    # rec[tok, h] = 1/max(den, eps); den sits at cols 128:130
                    # of each 130-wide pair slot. One strided op over all 3
                    # banks; slots 16:18 read unwritten psum (bank 2 has only
                    # 2 pairs) and are ignored.
                    rec = small.tile([P, 18], F32, tag="rec")
                    nc.vector.tensor_scalar_max(
                        rec[:].rearrange("p (b pr s) -> p b pr s",
                                         b=3, pr=3),
                        cps[:, :, 0:3 * PW].rearrange(
                            "p b (pr c) -> p b pr c",
                            c=PW)[:, :, :, P:P + 2],
                        EPS_DENOM)
                    nc.vector.reciprocal(rec[:, 0:16], rec[:, 0:16])

                    xn_t = work.tile([P, DIM], BF16, tag="xn")
                    nc.sync.dma_start(xn_t[:], xn_in[t0:t0 + P, :])

                    # y = ctx * rec (banks 0-1 in one strided op, bank 2 in
                    # another), then + x on gpsimd to keep DVE free
                    ym_t = work.tile([P, DIM], BF16, tag="ym")
                    src01 = cps[:, 0:2, 0:3 * PW].rearrange(
                        "p b (pr c) -> p b pr c", c=PW)[:, :, :, 0:P] \
                        .rearrange("p b pr (h d) -> p b pr h d", d=HD)
                    dst01 = ym_t[:, 0:768].rearrange(
                        "p (b pr h d) -> p b pr h d", b=2, pr=3, h=2)
                    rs01 = rec[:, 0:12].rearrange("p (b pr h) -> p b pr h",
                                                  b=2, h=2) \
                        .broadcast_to([P, 2, 3, 2, HD])
                    nc.vector.tensor_tensor(dst01, src01, rs01, ALU.mult)
                    src2 = cps[:, 2, 0:2 * PW].rearrange(
                        "p (pr c) -> p pr c", c=PW)[:, :, 0:P] \
                        .rearrange("p pr (h d) -> p pr h d", d=HD)
                    dst2 = ym_t[:, 768:1024].rearrange(
                        "p (pr h d) -> p pr h d", pr=2, h=2)
                    rs2 = rec[:, 12:16].rearrange("p (pr h) -> p pr h", h=2) \
                        .broadcast_to([P, 2, 2, HD])
                    nc.vector.tensor_tensor(dst2, src2, rs2, ALU.mult)

                    y_t = work.tile([P, DIM], BF16, tag="y")
                    nc.gpsimd.tensor_add(y_t[:], ym_t[:], xn_t[:])

                    # LayerNorm over channels (free dim)
                    stats = small.tile([P, 2, nc.vector.BN_STATS_DIM],
                                       F32, tag="stats")
                    mv = small.tile([P, nc.vector.BN_AGGR_DIM], F32,
                                    tag="mv")
                    yg = y_t[:].rearrange("p (g f) -> p g f", g=2)
                    for g in range(2):
                        nc.vector.bn_stats(stats[:, g, :], yg[:, g, :])
                    nc.vector.bn_aggr(mv[:], stats[:])
                    std = small.tile([P, 1], F32, tag="std")
                    nc.scalar.activation(std[:], mv[:, 1:2], AF.Sqrt,
                                         bias=eps_sb[:])
                    nc.vector.reciprocal(std[:], std[:])
                    # normalize on scalar: out = y*istd + (-mu*istd)
                    nbias = small.tile([P, 1], F32, tag="nbias")
                    nc.vector.tensor_scalar(nbias[:], mv[:, 0:1], std[:],
                                            -1.0, op0=ALU.mult, op1=ALU.mult)
                    out_t = work.tile([P, DIM], BF16, tag="out")
                    if with_affine:
                        nc.scalar.activation(out_t[:], y_t[:], AF.Identity,
                                             scale=std[:], bias=nbias[:])
                        nc.vector.tensor_mul(out_t[:], out_t[:], gamma_bc[:])
                        nc.gpsimd.tensor_add(out_t[:], out_t[:], beta_bc[:])
                    else:
                        nc.scalar.activation(out_t[:], y_t[:], AF.Identity,
                                             scale=std[:], bias=nbias[:])
                    nc.sync.dma_start(yn_out[t0:t0 + P, :], out_t[:])

    nc.compile()
    return nc


_CACHE: dict = {}


def _get_nc(with_kv_bias: bool = False, with_affine: bool = False):
    key = ("nc", with_kv_bias, with_affine)
    if key not in _CACHE:
        _CACHE[key] = build(with_kv_bias, with_affine)
    return _CACHE[key]


def make_in_maps(x, Wq, bq, Wk, bk, Wv, bv, gamma, beta):
    x = np.asarray(x, dtype=np.float32)
    f32 = lambda a: np.ascontiguousarray(np.asarray(a, dtype=np.float32))
    f8 = lambda a: np.ascontiguousarray(
        np.asarray(a, dtype=np.float32)).astype(ml_dtypes.float8_e4m3)
    bf = lambda a: np.ascontiguousarray(
        np.asarray(a, dtype=np.float32)).astype(ml_dtypes.bfloat16)
    wq8 = f8(np.asarray(Wq, np.float32).T * SW)
    wk8 = f8(np.asarray(Wk, np.float32).T * SW)
    wv8 = f8(np.asarray(Wv, np.float32).T * SW)
    bq, bk, bv = f32(bq), bf(bk), bf(bv)
    gamma, beta = bf(gamma), bf(beta)
    in_maps = []
    for c in range(N_CORES):
        b, half = divmod(c, 2)
        xs = x[b, half * T:(half + 1) * T, :]
        in_maps.append({
            "xt8": f8(xs.T * SX), "xn": bf(xs),
            "wq8": wq8, "wk8": wk8, "wv8": wv8,
            "bq": bq, "bk": bk, "bv": bv,
            "gamma": gamma, "beta": beta,
        })
    return in_maps


def kernel(x, Wq, bq, Wk, bk, Wv, bv, gamma, beta):
    with_kv_bias = bool(np.any(np.asarray(bk)) or np.any(np.asarray(bv)))
    with_affine = bool(np.any(np.asarray(beta))
                       or not np.all(np.asarray(gamma) == 1.0))
    nc = _get_nc(with_kv_bias, with_affine)
    in_maps = make_in_maps(x, Wq, bq, Wk, bk, Wv, bv, gamma, beta)
    res = run_bass_kernel_spmd(nc, in_maps, core_ids=list(range(N_CORES)))
    out = np.empty((B, NTOK, DIM), dtype=np.float32)
    for c in range(N_CORES):
        b, half = divmod(c, 2)
        out[b, half * T:(half + 1) * T, :] = \
            np.asarray(res.results[c]["yn"]).astype(np.float32)
    return out
